# revision 5
# baseline (speedup 1.0000x reference)
"""BankedLinear (MoE-style banked linear) Trainium2 kernel.

Reference computation (per token t, with k=2 selected banks):
    out[t] = sum_k prob[t,k] * (x[t] @ W[sel[t,k]] + bias[sel[t,k]])

Strategy (expert-parallel over 8 NeuronCores):
  - Core c owns banks [8c, 8c+8).  Its weight slab is the dominant,
    unavoidable HBM traffic; each bank is read exactly once system-wide.
  - Host routes token-bank pairs to cores by selected bank, pre-scales each
    gathered token row by its probability, transposes to [in_feature, slot],
    and pads to CAP=32 slots per bank.
  - Precision: harness tolerance is rel_err < 2e-2; a single bf16 matmul
    term (x_bf16 @ W_bf16 accumulated in fp32 PSUM) gives ~3e-3, so weights
    and activations ship as plain bf16 (half the fp32 bytes) and the
    per-slot results are stored as bf16 as well.
  - DMA orchestration: all weight DMAs are created contiguously so the
    8 round-robin DMAHW completion lanes cycle within the weight stream
    (interleaving compute-dependent stores into the lane sequence stalls
    the stream: lane reuse waits on the previous user's completion).
    Weights ride the sync-ring HWDGE in a wedge: 512KB slabs for banks
    0-5, halves for bank 6, quarters for bank 7, so the final matmul+copy+
    store tail is gated by a 128KB transfer instead of a 512KB one.
    The x dispatch and per-pair y stores ride the scalar-ring HWDGE.
  - Bias is folded in on the host (one gather + multiply-add over 1024
    pairs); host scatter-adds the per-pair device results into the output.

Fixed shapes: B=2, T=256, K=2, IN=OUT=512, NB=64 banks, 8 cores.
Capacity: 32 slots/bank (binomial mean 16, sd ~4; overflow pairs — none for
realistic routing — are handled exactly on the host as a fallback).
"""

import numpy as np
from contextlib import ExitStack

B, T, KSEL = 2, 256, 2
IN, OUT, NB = 512, 512, 64
NCORES = 8
BPC = NB // NCORES          # banks per core = 8
CAP = 32                    # padded token slots per bank
SLOTS = BPC * CAP           # 256 dispatch rows per core
PCHUNK = 128                # contraction chunk (SBUF partition dim)
KC = IN // PCHUNK           # 4 contraction chunks

_cache = {}

# kc chunks per weight DMA, per bank: wedge profile (see module docstring)
WKH = [4, 4, 4, 4, 4, 4, 2, 1]


def _build_nc():
    """Build the Bass/Tile program (one SPMD NeuronCore program)."""
    import concourse.tile as tile
    import concourse.mybir as mybir
    from concourse import bacc

    f32 = mybir.dt.float32
    bf16 = mybir.dt.bfloat16
    nc = bacc.Bacc("TRN2", target_bir_lowering=False, debug=False,
                   num_devices=NCORES)
    # host-pre-swizzled SBUF layouts: partition dim first, contiguous free dim
    xt = nc.dram_tensor("xt", [PCHUNK, KC * SLOTS], bf16,
                        kind="ExternalInput").ap()
    w = nc.dram_tensor("w", [BPC, PCHUNK, KC * OUT], bf16,
                       kind="ExternalInput").ap()
    y = nc.dram_tensor("y", [SLOTS, OUT], bf16, kind="ExternalOutput").ap()

    from concourse.tile import add_dep_helper

    def chain(dep_chain, binst, reason):
        # pin scheduler order: binst depends on the previous link
        if dep_chain:
            add_dep_helper(binst.ins, dep_chain[-1].ins, sync=False,
                           reason=reason)
        dep_chain.append(binst)

    with tile.TileContext(nc) as tc:
        with ExitStack() as ctx:
            xpool = ctx.enter_context(tc.tile_pool(name="xp", bufs=1))
            wpools = {
                kh: ctx.enter_context(
                    tc.tile_pool(name=f"wp{kh}",
                                 bufs=sum(KC // k for k in WKH if k == kh)))
                for kh in sorted(set(WKH))
            }
            ypool = ctx.enter_context(tc.tile_pool(name="yp", bufs=BPC // 2))
            pspool = ctx.enter_context(
                tc.tile_pool(name="ps", bufs=3, space="PSUM"))

            # token dispatch first on the sync ring: every matmul needs it
            xt_sb = xpool.tile([PCHUNK, KC * SLOTS], bf16, tag="xt")

            wq = []    # sync-ring DMA chain (keeps FIFO = weight order)
            sq = []    # scalar-ring chain: per-pair y stores
            mq = []    # PE matmul chain (keeps bank order = arrival order)
            chain(wq, nc.sync.dma_start(xt_sb[:], xt[:]), "xt first")

            # All weight DMAs created contiguously (lane hygiene, see above).
            wtiles = []            # wtiles[j] = list of tiles covering bank j
            for j in range(BPC):
                kh = WKH[j]
                tiles = []
                for i in range(KC // kh):
                    ks = slice(i * kh * OUT, (i + 1) * kh * OUT)
                    w_t = wpools[kh].tile([PCHUNK, kh * OUT], bf16,
                                          tag=f"w{kh}")
                    chain(wq, nc.sync.dma_start(w_t[:], w[j, :, ks]),
                          "weight ring order")
                    tiles.append(w_t)
                wtiles.append(tiles)

            # Banks processed in pairs. The even bank computes in PE column
            # group 0, the odd bank in column group 1 (tile_position), so
            # their matmuls coexist in the array. Each bank accumulates in
            # its OWN psum bank (separate tiles) so the per-bank start=True
            # has_written clear cannot disturb its neighbour.  Bank-major
            # matmul order: the even bank's weights always arrive first, so
            # its matmuls and PSUM->SBUF cast overlap the odd bank's stream.
            for p in range(BPC // 2):
                psA = pspool.tile([CAP, OUT], f32, tag="psA")
                psB = pspool.tile([2 * CAP, OUT], f32, tag="psB")
                outs = (psA[:], psB[CAP:2 * CAP, :])
                ysb = ypool.tile([2 * CAP, OUT], bf16, tag="y")
                for q in range(2):
                    j = 2 * p + q
                    kh = WKH[j]
                    for kc in range(KC):
                        xs = slice(kc * SLOTS + j * CAP,
                                   kc * SLOTS + (j + 1) * CAP)
                        wsl = wtiles[j][kc // kh][
                            :, (kc % kh) * OUT:(kc % kh + 1) * OUT]
                        mm = nc.tensor.matmul(
                            outs[q], xt_sb[:, xs], wsl,
                            start=(kc == 0), stop=(kc == KC - 1),
                            tile_position=(0, q * CAP),
                            skip_group_check=True)
                        if kc == 0 and q == 0:
                            chain(mq, mm, "pair compute order")
                    # per-bank cast: the even bank's cast overlaps the odd
                    # bank's matmuls
                    nc.vector.tensor_copy(ysb[q * CAP:(q + 1) * CAP, :],
                                          outs[q])
                # eager per-pair store on the scalar ring: overlaps the
                # remaining weight stream instead of queuing behind it.
                # The first store is order-pinned after the last weight DMA
                # so the scheduler cannot interleave stores into the weight
                # stream's DMAHW lane cycle (lane reuse waits on the previous
                # user's completion, which would stall the weight stream).
                ydma = nc.scalar.dma_start(
                    y[p * 2 * CAP:(p + 1) * 2 * CAP, :], ysb[:])
                if not sq:
                    add_dep_helper(ydma.ins, wq[-1].ins, sync=False,
                                   reason="stores after weight lane cycle")
                chain(sq, ydma, "y store order")
    nc.compile()
    return nc


def _get_nc():
    if "nc" not in _cache:
        _cache["nc"] = _build_nc()
    return _cache["nc"]


def _bf16(a32):
    import ml_dtypes
    return a32.astype(ml_dtypes.bfloat16)


def _swizzle_x(xt):
    """[IN, SLOTS] -> [128, KC*SLOTS] with free index (kc, slot)."""
    return np.ascontiguousarray(
        xt.reshape(KC, PCHUNK, SLOTS).transpose(1, 0, 2).reshape(
            PCHUNK, KC * SLOTS))


def _swizzle_w(w):
    """[BPC, IN, OUT] -> [BPC, 128, KC*OUT] with free index (kc, out)."""
    return np.ascontiguousarray(
        w.reshape(BPC, KC, PCHUNK, OUT).transpose(0, 2, 1, 3).reshape(
            BPC, PCHUNK, KC * OUT))


def _route(X, sel, prob):
    """Group token-bank pairs by bank, build per-core dispatch arrays.

    Returns (slot_tok [NCORES,SLOTS] int64 (-1=pad), slot_p, overflow list
    of (token, bank, prob))."""
    NT = X.shape[0]
    pair_tok = np.repeat(np.arange(NT, dtype=np.int64), KSEL)
    pair_bank = sel.reshape(-1)
    pair_p = prob.reshape(-1)

    order = np.argsort(pair_bank, kind="stable")
    counts = np.bincount(pair_bank, minlength=NB)
    starts = np.concatenate(([0], np.cumsum(counts)))

    slot_tok = np.full((NCORES, SLOTS), -1, dtype=np.int64)
    slot_p = np.zeros((NCORES, SLOTS), dtype=np.float32)
    overflow = []
    for b in range(NB):
        c, j = divmod(b, BPC)
        s0, s1 = starts[b], starts[b + 1]
        take = min(s1 - s0, CAP)
        idx = order[s0:s0 + take]
        slot_tok[c, j * CAP: j * CAP + take] = pair_tok[idx]
        slot_p[c, j * CAP: j * CAP + take] = pair_p[idx]
        for i in order[s0 + take:s1]:
            overflow.append((int(pair_tok[i]), b, float(pair_p[i])))
    return slot_tok, slot_p, overflow


def _combine(ys, slot_tok, X, sel, prob, weights, bias, overflow):
    NT = X.shape[0]
    out = np.zeros((NT, OUT), dtype=np.float32)
    for c in range(NCORES):
        tok = slot_tok[c]
        valid = tok >= 0
        np.add.at(out, tok[valid], ys[c].astype(np.float32)[valid])
    # bias term for every pair (device computes x @ W only)
    for k in range(KSEL):
        out += prob[:, k, None] * bias[sel[:, k]]
    # exact host fallback for capacity-overflow pairs (expected: none)
    for t, b, p in overflow:
        out[t] += p * (X[t] @ weights[b])
    return out


def _run_device(in_maps, trace=False, **kwargs):
    from concourse.bass_utils import run_bass_kernel_spmd
    return run_bass_kernel_spmd(_get_nc(), in_maps,
                                core_ids=list(range(NCORES)),
                                trace=trace, **kwargs)


def kernel(_trace=False, _bass_results=None, **inputs):
    tensor = np.asarray(inputs["tensor"], dtype=np.float32)
    sel = np.asarray(inputs["bank_selections"]).astype(np.int64)
    prob = np.asarray(inputs["bank_probabilities"], dtype=np.float32)
    weights = np.asarray(inputs["weights"], dtype=np.float32)
    bias = np.asarray(inputs["bias"], dtype=np.float32)

    NT = tensor.shape[0] * tensor.shape[1]
    X = tensor.reshape(NT, IN)
    sel2 = sel.reshape(NT, KSEL)
    prob2 = prob.reshape(NT, KSEL)

    slot_tok, slot_p, overflow = _route(X, sel2, prob2)

    in_maps = []
    for c in range(NCORES):
        tok = slot_tok[c]
        rows = X[np.where(tok >= 0, tok, 0)] * slot_p[c][:, None]
        xt = np.ascontiguousarray(rows.T)              # [IN, SLOTS] fp32
        w32 = weights[c * BPC:(c + 1) * BPC]           # (8, 512, 512) fp32
        in_maps.append({
            "xt": _swizzle_x(_bf16(xt)),
            "w": _swizzle_w(_bf16(w32)),
        })

    res = _run_device(in_maps, trace=_trace)
    if _bass_results is not None:
        _bass_results.append(res)
    ys = [res.results[c]["y"] for c in range(NCORES)]

    out = _combine(ys, slot_tok, X, sel2, prob2, weights, bias, overflow)
    return out.reshape(tensor.shape[0], tensor.shape[1], OUT)


# revision 7
# speedup vs baseline: 1.0186x; 1.0186x over previous
"""BankedLinear (MoE-style banked linear) Trainium2 kernel.

Reference computation (per token t, with k=2 selected banks):
    out[t] = sum_k prob[t,k] * (x[t] @ W[sel[t,k]] + bias[sel[t,k]])

Strategy (expert-parallel over 8 NeuronCores):
  - Core c owns banks [8c, 8c+8).  Its weight slab is the dominant,
    unavoidable HBM traffic; each bank is read exactly once system-wide.
  - Host routes token-bank pairs to cores by selected bank, pre-scales each
    gathered token row by its probability, transposes to [in_feature, slot],
    and pads to CAP=32 slots per bank.
  - Precision: harness tolerance is rel_err < 2e-2; a single bf16 matmul
    term (x_bf16 @ W_bf16 accumulated in fp32 PSUM) gives ~3e-3, so weights
    and activations ship as plain bf16 (half the fp32 bytes) and the
    per-slot results are stored as bf16 as well.
  - DMA orchestration: all weight DMAs are created contiguously so the
    8 round-robin DMAHW completion lanes cycle within the weight stream
    (interleaving compute-dependent stores into the lane sequence stalls
    the stream: lane reuse waits on the previous user's completion).
    Weights ride the sync-ring HWDGE in a wedge: 512KB slabs for banks
    0-5, halves for bank 6, quarters for bank 7, so the final matmul+copy+
    store tail is gated by a 128KB transfer instead of a 512KB one.
    The x dispatch and per-pair y stores ride the scalar-ring HWDGE.
  - Bias is folded in on the host (one gather + multiply-add over 1024
    pairs); host scatter-adds the per-pair device results into the output.

Fixed shapes: B=2, T=256, K=2, IN=OUT=512, NB=64 banks, 8 cores.
Capacity: 32 slots/bank (binomial mean 16, sd ~4; overflow pairs — none for
realistic routing — are handled exactly on the host as a fallback).
"""

import numpy as np
from contextlib import ExitStack

B, T, KSEL = 2, 256, 2
IN, OUT, NB = 512, 512, 64
NCORES = 8
BPC = NB // NCORES          # banks per core = 8
CAP = 32                    # padded token slots per bank
SLOTS = BPC * CAP           # 256 dispatch rows per core
PCHUNK = 128                # contraction chunk (SBUF partition dim)
KC = IN // PCHUNK           # 4 contraction chunks

_cache = {}

# kc chunks per weight DMA, per bank: wedge profile at both ends — small
# chunks for bank 0 (first matmul gates on a 128KB completion instead of
# 512KB) and for bank 7 (the end-of-stream tail is gated by 128KB), big
# slabs in the middle (descriptor-generation efficiency)
WKH = [1, 2, 4, 4, 4, 4, 2, 1]


def _build_nc():
    """Build the Bass/Tile program (one SPMD NeuronCore program)."""
    import concourse.tile as tile
    import concourse.mybir as mybir
    from concourse import bacc

    f32 = mybir.dt.float32
    bf16 = mybir.dt.bfloat16
    nc = bacc.Bacc("TRN2", target_bir_lowering=False, debug=False,
                   num_devices=NCORES)
    # host-pre-swizzled SBUF layouts: partition dim first, contiguous free dim
    xt = nc.dram_tensor("xt", [PCHUNK, KC * SLOTS], bf16,
                        kind="ExternalInput").ap()
    w = nc.dram_tensor("w", [BPC, PCHUNK, KC * OUT], bf16,
                       kind="ExternalInput").ap()
    y = nc.dram_tensor("y", [SLOTS, OUT], bf16, kind="ExternalOutput").ap()

    from concourse.tile import add_dep_helper

    def chain(dep_chain, binst, reason):
        # pin scheduler order: binst depends on the previous link
        if dep_chain:
            add_dep_helper(binst.ins, dep_chain[-1].ins, sync=False,
                           reason=reason)
        dep_chain.append(binst)

    with tile.TileContext(nc) as tc:
        with ExitStack() as ctx:
            xpool = ctx.enter_context(tc.tile_pool(name="xp", bufs=1))
            wpools = {
                kh: ctx.enter_context(
                    tc.tile_pool(name=f"wp{kh}",
                                 bufs=sum(KC // k for k in WKH if k == kh)))
                for kh in sorted(set(WKH))
            }
            ypool = ctx.enter_context(tc.tile_pool(name="yp", bufs=BPC // 2))
            pspool = ctx.enter_context(
                tc.tile_pool(name="ps", bufs=4, space="PSUM"))

            # token dispatch first on the sync ring: every matmul needs it
            xt_sb = xpool.tile([PCHUNK, KC * SLOTS], bf16, tag="xt")

            wq = []    # sync-ring DMA chain (keeps FIFO = weight order)
            sq = []    # scalar-ring chain: per-pair y stores
            mq = []    # PE matmul chain (keeps bank order = arrival order)
            chain(wq, nc.sync.dma_start(xt_sb[:], xt[:]), "xt first")

            # All weight DMAs created contiguously (lane hygiene, see above).
            wtiles = []            # wtiles[j] = list of tiles covering bank j
            for j in range(BPC):
                kh = WKH[j]
                tiles = []
                for i in range(KC // kh):
                    ks = slice(i * kh * OUT, (i + 1) * kh * OUT)
                    w_t = wpools[kh].tile([PCHUNK, kh * OUT], bf16,
                                          tag=f"w{kh}")
                    chain(wq, nc.sync.dma_start(w_t[:], w[j, :, ks]),
                          "weight ring order")
                    tiles.append(w_t)
                wtiles.append(tiles)

            # Banks processed in pairs. The even bank computes in PE column
            # group 0, the odd bank in column group 1 (tile_position), so
            # their matmuls coexist in the array. Each bank accumulates in
            # its OWN psum bank (separate tiles) so the per-bank start=True
            # has_written clear cannot disturb its neighbour.  Bank-major
            # matmul order: the even bank's weights always arrive first, so
            # its matmuls and PSUM->SBUF cast overlap the odd bank's stream.
            for p in range(BPC // 2):
                psA = pspool.tile([CAP, OUT], f32, tag="psA")
                psB = pspool.tile([2 * CAP, OUT], f32, tag="psB")
                outs = (psA[:], psB[CAP:2 * CAP, :])
                ysb = ypool.tile([2 * CAP, OUT], bf16, tag="y")
                for q in range(2):
                    j = 2 * p + q
                    kh = WKH[j]
                    for kc in range(KC):
                        xs = slice(kc * SLOTS + j * CAP,
                                   kc * SLOTS + (j + 1) * CAP)
                        wsl = wtiles[j][kc // kh][
                            :, (kc % kh) * OUT:(kc % kh + 1) * OUT]
                        mm = nc.tensor.matmul(
                            outs[q], xt_sb[:, xs], wsl,
                            start=(kc == 0), stop=(kc == KC - 1),
                            tile_position=(0, q * CAP),
                            skip_group_check=True)
                        if kc == 0 and q == 0:
                            chain(mq, mm, "pair compute order")
                    # per-bank cast: the even bank's cast overlaps the odd
                    # bank's matmuls
                    nc.vector.tensor_copy(ysb[q * CAP:(q + 1) * CAP, :],
                                          outs[q])
                # eager per-pair store on the scalar ring: overlaps the
                # remaining weight stream instead of queuing behind it.
                # The first store is order-pinned after the last weight DMA
                # so the scheduler cannot interleave stores into the weight
                # stream's DMAHW lane cycle (lane reuse waits on the previous
                # user's completion, which would stall the weight stream).
                ydma = nc.scalar.dma_start(
                    y[p * 2 * CAP:(p + 1) * 2 * CAP, :], ysb[:])
                if not sq:
                    add_dep_helper(ydma.ins, wq[-1].ins, sync=False,
                                   reason="stores after weight lane cycle")
                chain(sq, ydma, "y store order")
    nc.compile()
    return nc


def _get_nc():
    if "nc" not in _cache:
        _cache["nc"] = _build_nc()
    return _cache["nc"]


def _bf16(a32):
    import ml_dtypes
    return a32.astype(ml_dtypes.bfloat16)


def _swizzle_x(xt):
    """[IN, SLOTS] -> [128, KC*SLOTS] with free index (kc, slot)."""
    return np.ascontiguousarray(
        xt.reshape(KC, PCHUNK, SLOTS).transpose(1, 0, 2).reshape(
            PCHUNK, KC * SLOTS))


def _swizzle_w(w):
    """[BPC, IN, OUT] -> [BPC, 128, KC*OUT] with free index (kc, out)."""
    return np.ascontiguousarray(
        w.reshape(BPC, KC, PCHUNK, OUT).transpose(0, 2, 1, 3).reshape(
            BPC, PCHUNK, KC * OUT))


def _route(X, sel, prob):
    """Group token-bank pairs by bank, build per-core dispatch arrays.

    Returns (slot_tok [NCORES,SLOTS] int64 (-1=pad), slot_p, overflow list
    of (token, bank, prob))."""
    NT = X.shape[0]
    pair_tok = np.repeat(np.arange(NT, dtype=np.int64), KSEL)
    pair_bank = sel.reshape(-1)
    pair_p = prob.reshape(-1)

    order = np.argsort(pair_bank, kind="stable")
    counts = np.bincount(pair_bank, minlength=NB)
    starts = np.concatenate(([0], np.cumsum(counts)))

    slot_tok = np.full((NCORES, SLOTS), -1, dtype=np.int64)
    slot_p = np.zeros((NCORES, SLOTS), dtype=np.float32)
    overflow = []
    for b in range(NB):
        c, j = divmod(b, BPC)
        s0, s1 = starts[b], starts[b + 1]
        take = min(s1 - s0, CAP)
        idx = order[s0:s0 + take]
        slot_tok[c, j * CAP: j * CAP + take] = pair_tok[idx]
        slot_p[c, j * CAP: j * CAP + take] = pair_p[idx]
        for i in order[s0 + take:s1]:
            overflow.append((int(pair_tok[i]), b, float(pair_p[i])))
    return slot_tok, slot_p, overflow


def _combine(ys, slot_tok, X, sel, prob, weights, bias, overflow):
    NT = X.shape[0]
    out = np.zeros((NT, OUT), dtype=np.float32)
    for c in range(NCORES):
        tok = slot_tok[c]
        valid = tok >= 0
        np.add.at(out, tok[valid], ys[c].astype(np.float32)[valid])
    # bias term for every pair (device computes x @ W only)
    for k in range(KSEL):
        out += prob[:, k, None] * bias[sel[:, k]]
    # exact host fallback for capacity-overflow pairs (expected: none)
    for t, b, p in overflow:
        out[t] += p * (X[t] @ weights[b])
    return out


def _run_device(in_maps, trace=False, **kwargs):
    from concourse.bass_utils import run_bass_kernel_spmd
    return run_bass_kernel_spmd(_get_nc(), in_maps,
                                core_ids=list(range(NCORES)),
                                trace=trace, **kwargs)


def kernel(_trace=False, _bass_results=None, **inputs):
    tensor = np.asarray(inputs["tensor"], dtype=np.float32)
    sel = np.asarray(inputs["bank_selections"]).astype(np.int64)
    prob = np.asarray(inputs["bank_probabilities"], dtype=np.float32)
    weights = np.asarray(inputs["weights"], dtype=np.float32)
    bias = np.asarray(inputs["bias"], dtype=np.float32)

    NT = tensor.shape[0] * tensor.shape[1]
    X = tensor.reshape(NT, IN)
    sel2 = sel.reshape(NT, KSEL)
    prob2 = prob.reshape(NT, KSEL)

    slot_tok, slot_p, overflow = _route(X, sel2, prob2)

    in_maps = []
    for c in range(NCORES):
        tok = slot_tok[c]
        rows = X[np.where(tok >= 0, tok, 0)] * slot_p[c][:, None]
        xt = np.ascontiguousarray(rows.T)              # [IN, SLOTS] fp32
        w32 = weights[c * BPC:(c + 1) * BPC]           # (8, 512, 512) fp32
        in_maps.append({
            "xt": _swizzle_x(_bf16(xt)),
            "w": _swizzle_w(_bf16(w32)),
        })

    res = _run_device(in_maps, trace=_trace)
    if _bass_results is not None:
        _bass_results.append(res)
    ys = [res.results[c]["y"] for c in range(NCORES)]

    out = _combine(ys, slot_tok, X, sel2, prob2, weights, bias, overflow)
    return out.reshape(tensor.shape[0], tensor.shape[1], OUT)
